# revision 1
# baseline (speedup 1.0000x reference)
"""LDDMM variational shooting RHS on 8 Trainium2 NeuronCores.

reference math (B=1, N=8192, D=3, sigma=0.1):
    p   = clip(mom, -1, 1)
    d2  = |x_i - x_j|^2
    K   = exp(-d2 / (2 sig^2)) = exp(-50 d2)
    dcp = K @ p
    W   = K * (p p^T)
    row = W @ 1;  Wx = W @ x
    dmom = (1/sig^2) (x * row - Wx)

Device strategy (row-sharded over 8 cores, 1024 rows each):
  - work in transposed tiles Kt[j, i] (j on SBUF partitions) so both the
    d2 generation and the j-contraction map onto the tensor engine.
  - d2 gen: single K_dim=13 fp16 matmul per [128j x 512i] tile using hi/lo
    split operands (fp16 streams 1 cycle/row; fp32 would cost 4x):
      d2[j,i] = sq_j + sq_i - 2(xh_j xh_i + xl_j xh_i + xh_j xl_i)
  - exp on the scalar (ACT) engine, PSUM -> SBUF fp16, grouped 3 tiles per
    instruction to amortize the per-instruction overhead.
  - everything downstream of K is one accumulating matmul with
      R = [p | vec(p (x) x)]  in R^{N x 12}:   S[m, i] = sum_j Kt[j,i] R[j,m]
    because  dcp_i = S[0:3, i],  row_i = p_i . dcp_i,
             (W x)_ie = sum_d p_id S[3+3d+e, i].
  - tiny host postprocess of S -> (dmom, dcp).
"""

import os
import sys

import numpy as np

if "/opt/trn_rl_repo" not in sys.path:
    sys.path.insert(0, "/opt/trn_rl_repo")

SIG2 = 0.01
N = 8192
D = 3
NCORES = 8
RPC = N // NCORES          # rows (i) per core = 1024
ICHUNK = 512               # i columns per matmul (one PSUM bank, fp32 out)
NIB = RPC // ICHUNK        # i-chunks per core = 2
JTILE = 128                # j rows per tile (PE contraction dim)
NJT = N // JTILE           # 64 j-tiles
GROUP = 3                  # j-tiles per ACT instruction (3 PSUM banks)
KDIM = 13                  # gen matmul contraction dim (hi/lo split)
RCOLS = 12                 # reduction matrix columns

_cache: dict = {}

# Program variant. "asym" (production): alternating 4-bank/3-bank exp
# groups across two PSUM pools — bigger ACT calls amortize per-instruction
# overhead, pool alternation double-buffers so PE gen overlaps ACT.
# Others are benchmarking ablations ("full"=uniform 3-bank groups,
# "genact"=no reductions, "gen"=matmuls only, "g2b3", "packed"=row-banded
# concurrent gen matmuls -- crashes the exec unit, kept for reference).
VARIANT = "asym"

# last BassKernelResults (exec_time_ns etc.) for the test harness
last_result = None


def _build_program(loop_m: int = 1):
    """Build (once) the Bass/Tile program shared by all 8 cores.

    loop_m > 1 unrolls the whole computation M times inside one NEFF —
    used only by the benchmarking harness to measure steady-state
    per-iteration device time through the axon dispatch overhead.
    """
    import concourse.bass as bass  # noqa: F401
    import concourse.mybir as mybir
    import concourse.tile as tile
    from concourse import bacc

    dt = mybir.dt
    nc = bacc.Bacc("TRN2", target_bir_lowering=False, debug=False)

    packed = VARIANT == "packed"
    nband = GROUP if packed else 1
    Ah = nc.dram_tensor("a_gen", [nband * KDIM, N], dt.float16,
                        kind="ExternalInput")
    Bh = nc.dram_tensor("b_gen", [nband * KDIM, RPC], dt.float16,
                        kind="ExternalInput")
    Rh = nc.dram_tensor("r_red", [JTILE, NJT * RCOLS], dt.float16,
                        kind="ExternalInput")
    So = nc.dram_tensor("s_out", [RCOLS, RPC], dt.float32,
                        kind="ExternalOutput")

    if VARIANT == "asym":
        # alternating 4/3-bank groups, each size a distinct pool so
        # consecutive groups double-buffer across the two pools
        sizes = []
        while sum(sizes) + 7 <= NJT:
            sizes += [4, 3]
        while sum(sizes) < NJT:
            sizes.append(min(4, NJT - sum(sizes)))
    elif VARIANT == "g2b3":
        sizes = [2] * (NJT // 2)
    else:
        sizes = [GROUP] * (NJT // GROUP)
        if NJT % GROUP:
            sizes.append(NJT % GROUP)
    groups = []
    jt = 0
    for s in sizes:
        groups.append(list(range(jt, jt + s)))
        jt += s

    if VARIANT == "asym":
        d2_cfg = [("d2a", 1, 4), ("d2b", 1, 3)]   # (name, bufs, banks)
        sacc_bufs = 1
    elif VARIANT == "g2b3":
        d2_cfg = [("d2", 3, 2)]
        sacc_bufs = 2
    else:
        d2_cfg = [("d2", 2, GROUP)]
        sacc_bufs = 2

    with tile.TileContext(nc) as tc:
        import contextlib
        with contextlib.ExitStack() as stack:
            cpool = stack.enter_context(tc.tile_pool(name="const", bufs=1))
            kpool = stack.enter_context(tc.tile_pool(name="ksb", bufs=3))
            spool = stack.enter_context(tc.tile_pool(name="ssb", bufs=2))
            d2pools = [
                (stack.enter_context(
                    tc.tile_pool(name=nm, bufs=bf, space="PSUM")), banks)
                for nm, bf, banks in d2_cfg
            ]
            sapool = stack.enter_context(
                tc.tile_pool(name="sacc", bufs=sacc_bufs, space="PSUM"))
            if packed:
                # bands of 13 weight rows at partitions {0,32,64} so the
                # three gen matmuls of a group occupy disjoint PE row
                # groups and run concurrently (tile_position auto-derives
                # from base_partition).
                a_sb = cpool.tile([64 + KDIM, N], dt.float16)
                b_sb = cpool.tile([64 + KDIM, RPC], dt.float16)
                for b in range(GROUP):
                    nc.sync.dma_start(
                        out=a_sb[32 * b:32 * b + KDIM, :],
                        in_=Ah.ap()[KDIM * b:KDIM * (b + 1), :])
                    nc.sync.dma_start(
                        out=b_sb[32 * b:32 * b + KDIM, :],
                        in_=Bh.ap()[KDIM * b:KDIM * (b + 1), :])
            else:
                a_sb = cpool.tile([KDIM, N], dt.float16)
                b_sb = cpool.tile([KDIM, RPC], dt.float16)
                nc.sync.dma_start(out=a_sb, in_=Ah.ap())
                nc.sync.dma_start(out=b_sb, in_=Bh.ap())
            r_sb = cpool.tile([JTILE, NJT * RCOLS], dt.float16)
            nc.sync.dma_start(out=r_sb, in_=Rh.ap())

            for ib in [i % NIB for i in range(NIB * loop_m)]:
                s_ps = sapool.tile([RCOLS, ICHUNK], dt.float32)
                isl = slice(ib * ICHUNK, (ib + 1) * ICHUNK)

                pending = None  # (jts, k_sb) whose reduction is not yet emitted
                for gi, jts in enumerate(groups):
                    w = len(jts) * ICHUNK
                    pool, banks = d2pools[gi % len(d2pools)]
                    d2 = pool.tile([JTILE, banks * ICHUNK], dt.float32)
                    for idx, jt in enumerate(jts):
                        if packed:
                            lhsT = a_sb[32 * idx:32 * idx + KDIM,
                                        jt * JTILE:(jt + 1) * JTILE]
                            rhs = b_sb[32 * idx:32 * idx + KDIM, isl]
                        else:
                            lhsT = a_sb[:, jt * JTILE:(jt + 1) * JTILE]
                            rhs = b_sb[:, isl]
                        nc.tensor.matmul(
                            d2[:, idx * ICHUNK:(idx + 1) * ICHUNK],
                            lhsT, rhs,
                            start=True, stop=True,
                        )
                    # software pipeline: emit previous group's reductions
                    # between this group's gen and exp so the PE never FIFO
                    # blocks behind a reduction waiting on the ACT engine.
                    if pending is not None and VARIANT not in ("gen", "genact"):
                        pjts, pk = pending
                        for idx, jt in enumerate(pjts):
                            nc.tensor.matmul(
                                s_ps,
                                r_sb[:, jt * RCOLS:(jt + 1) * RCOLS],
                                pk[:, idx * ICHUNK:(idx + 1) * ICHUNK],
                                start=(jt == 0), stop=(jt == NJT - 1),
                            )
                    if VARIANT != "gen":
                        k_sb = kpool.tile([JTILE, banks * ICHUNK], dt.float16)
                        nc.scalar.activation(
                            k_sb[:, :w], d2[:, :w],
                            mybir.ActivationFunctionType.Exp,
                            scale=-1.0 / (2.0 * SIG2),
                        )
                        pending = (jts, k_sb)

                if VARIANT not in ("gen", "genact"):
                    pjts, pk = pending
                    for idx, jt in enumerate(pjts):
                        nc.tensor.matmul(
                            s_ps,
                            r_sb[:, jt * RCOLS:(jt + 1) * RCOLS],
                            pk[:, idx * ICHUNK:(idx + 1) * ICHUNK],
                            start=(jt == 0), stop=(jt == NJT - 1),
                        )

                s_out = spool.tile([RCOLS, ICHUNK], dt.float32)
                if VARIANT not in ("gen", "genact"):
                    nc.vector.tensor_copy(s_out, s_ps)
                else:
                    nc.vector.memset(s_out, 0.0)
                nc.sync.dma_start(
                    out=So.ap()[:, ib * ICHUNK:(ib + 1) * ICHUNK], in_=s_out
                )

    nc.compile()
    return nc


def _split_hi_lo(v32: np.ndarray):
    """fp32 -> (hi, lo) float16 pair with v ~= hi + lo."""
    hi = v32.astype(np.float16)
    lo = (v32 - hi.astype(np.float32)).astype(np.float16)
    return hi, lo


def _host_prep(mom: np.ndarray, control_points: np.ndarray):
    x = np.asarray(control_points, np.float32).reshape(N, D)
    p = np.clip(np.asarray(mom, np.float32).reshape(N, D), -1.0, 1.0)

    sq = np.sum(x.astype(np.float64) * x.astype(np.float64), axis=1)
    sq = sq.astype(np.float32)
    xh, xl = _split_hi_lo(x)
    sqh, sql = _split_hi_lo(sq)
    ones = np.ones(N, np.float16)

    # lhsT (stationary, per-j): 13 rows
    A = np.empty((KDIM, N), np.float16)
    A[0:3] = xh.T
    A[3:6] = xl.T
    A[6:9] = xh.T
    A[9] = sqh
    A[10] = sql
    A[11] = ones
    A[12] = ones

    # rhs (moving, per-i): 13 rows
    m2xh = (-2.0 * xh.astype(np.float32)).astype(np.float16)
    m2xl = (-2.0 * xl.astype(np.float32)).astype(np.float16)
    Bfull = np.empty((KDIM, N), np.float16)
    Bfull[0:3] = m2xh.T
    Bfull[3:6] = m2xh.T
    Bfull[6:9] = m2xl.T
    Bfull[9] = ones
    Bfull[10] = ones
    Bfull[11] = sqh
    Bfull[12] = sql

    # reduction matrix R = [p | vec(p (x) x)], packed [128, 64*12]
    R = np.empty((N, RCOLS), np.float32)
    R[:, 0:3] = p
    R[:, 3:12] = (p[:, :, None] * x[:, None, :]).reshape(N, 9)
    Rp = (
        R.reshape(NJT, JTILE, RCOLS)
        .transpose(1, 0, 2)
        .reshape(JTILE, NJT * RCOLS)
        .astype(np.float16)
    )
    return x, p, A, Bfull, Rp


def build_in_maps(A, Bfull, Rp):
    nband = GROUP if VARIANT == "packed" else 1
    At = np.ascontiguousarray(np.tile(A, (nband, 1)))
    in_maps = []
    for c in range(NCORES):
        Bc = Bfull[:, c * RPC:(c + 1) * RPC]
        in_maps.append({
            "a_gen": At,
            "b_gen": np.ascontiguousarray(np.tile(Bc, (nband, 1))),
            "r_red": Rp,
        })
    return in_maps


def kernel(mom: np.ndarray, control_points: np.ndarray):
    global last_result
    from concourse.bass_utils import run_bass_kernel_spmd

    x, p, A, Bfull, Rp = _host_prep(mom, control_points)

    loop_m = int(os.environ.get("KERNEL_LOOP_M", "1"))
    key = ("nc", loop_m, VARIANT)
    if key not in _cache:
        _cache[key] = _build_program(loop_m)
    nc = _cache[key]

    in_maps = build_in_maps(A, Bfull, Rp)

    trace = os.environ.get("KERNEL_TRACE", "0") == "1"
    res = run_bass_kernel_spmd(
        nc, in_maps, core_ids=list(range(NCORES)), trace=trace,
    )
    last_result = res

    S = np.concatenate([r["s_out"] for r in res.results], axis=1)  # [12, N]

    dcp = S[0:3].T                                   # [N, 3]
    row = np.einsum("nd,dn->n", p, S[0:3])           # p_i . (K p)_i
    Wx = np.einsum("nd,den->ne", p, S[3:12].reshape(D, D, N))
    dmom = (1.0 / SIG2) * (x * row[:, None] - Wx)

    return (
        dmom.reshape(1, N, D).astype(np.float32),
        dcp.reshape(1, N, D).astype(np.float32),
    )



# revision 7
# speedup vs baseline: 742.4435x; 742.4435x over previous
"""LDDMM variational shooting RHS on 8 Trainium2 NeuronCores.

reference math (B=1, N=8192, D=3, sigma=0.1):
    p   = clip(mom, -1, 1)
    d2  = |x_i - x_j|^2
    K   = exp(-d2 / (2 sig^2)) = exp(-50 d2)
    dcp = K @ p
    W   = K * (p p^T)
    row = W @ 1;  Wx = W @ x
    dmom = (1/sig^2) (x * row - Wx)

Device strategy (row-sharded over 8 cores, 1024 rows each):
  - work in transposed tiles Kt[j, i] (j on SBUF partitions) so both the
    d2 generation and the j-contraction map onto the tensor engine.
  - d2 gen: single K_dim=13 fp16 matmul per [128j x 512i] tile using hi/lo
    split operands (fp16 streams 1 cycle/row; fp32 would cost 4x):
      d2[j,i] = sq_j + sq_i - 2(xh_j xh_i + xl_j xh_i + xh_j xl_i)
  - exp on the scalar (ACT) engine, PSUM -> SBUF fp16, grouped 3 tiles per
    instruction to amortize the per-instruction overhead.
  - everything downstream of K is one accumulating matmul with
      R = [p | vec(p (x) x)]  in R^{N x 12}:   S[m, i] = sum_j Kt[j,i] R[j,m]
    because  dcp_i = S[0:3, i],  row_i = p_i . dcp_i,
             (W x)_ie = sum_d p_id S[3+3d+e, i].
  - tiny host postprocess of S -> (dmom, dcp).
"""

import os
import sys

import numpy as np

if "/opt/trn_rl_repo" not in sys.path:
    sys.path.insert(0, "/opt/trn_rl_repo")

SIG2 = 0.01
N = 8192
D = 3
NCORES = 8
RPC = N // NCORES          # rows (i) per core = 1024
ICHUNK = 512               # i columns per matmul (one PSUM bank, fp32 out)
NIB = RPC // ICHUNK        # i-chunks per core = 2
JTILE = 128                # j rows per tile (PE contraction dim)
NJT = N // JTILE           # 64 j-tiles
GROUP = 3                  # j-tiles per ACT instruction (3 PSUM banks)
KDIM = 13                  # gen matmul contraction dim (hi/lo split)
RCOLS = 12                 # reduction matrix columns

_cache: dict = {}

# Program variant. "sym" (production): symmetric block-pair schedule —
# each core computes K tiles for j-blocks c..c+4 against its own i-block c
# (5/8 of the dense exp work), reduces them forward, and reuses slots 1..3
# via hardware DMA (XBAR) transposes to emit mirror contributions for row
# blocks c+1..c+3.  Host sums the per-core partial S matrices.
# "asym" is the previous dense row-sharded kernel; others are ablations.
VARIANT = "sym"

SLOTS = 5          # j-block offsets covered forward (0..4)
MIR_SLOTS = (1, 2, 3)  # slots whose tiles are reused transposed
NBLK = 8           # row blocks (one per core)
BLK = N // NBLK    # 1024 rows per block
NJT_B = BLK // JTILE   # 8 j-tiles per block

# last BassKernelResults (exec_time_ns etc.) for the test harness
last_result = None


def _build_program(loop_m: int = 1):
    """Build (once) the Bass/Tile program shared by all 8 cores.

    loop_m > 1 unrolls the whole computation M times inside one NEFF —
    used only by the benchmarking harness to measure steady-state
    per-iteration device time through the axon dispatch overhead.
    """
    import concourse.bass as bass  # noqa: F401
    import concourse.mybir as mybir
    import concourse.tile as tile
    from concourse import bacc

    dt = mybir.dt
    nc = bacc.Bacc("TRN2", target_bir_lowering=False, debug=False)

    packed = VARIANT == "packed"
    nband = GROUP if packed else 1
    Ah = nc.dram_tensor("a_gen", [nband * KDIM, N], dt.float16,
                        kind="ExternalInput")
    Bh = nc.dram_tensor("b_gen", [nband * KDIM, RPC], dt.float16,
                        kind="ExternalInput")
    Rh = nc.dram_tensor("r_red", [JTILE, NJT * RCOLS], dt.float16,
                        kind="ExternalInput")
    So = nc.dram_tensor("s_out", [RCOLS, RPC], dt.float32,
                        kind="ExternalOutput")

    if VARIANT == "asym":
        # alternating 4/3-bank groups, each size a distinct pool so
        # consecutive groups double-buffer across the two pools
        sizes = []
        while sum(sizes) + 7 <= NJT:
            sizes += [4, 3]
        while sum(sizes) < NJT:
            sizes.append(min(4, NJT - sum(sizes)))
    elif VARIANT == "g2b3":
        sizes = [2] * (NJT // 2)
    else:
        sizes = [GROUP] * (NJT // GROUP)
        if NJT % GROUP:
            sizes.append(NJT % GROUP)
    groups = []
    jt = 0
    for s in sizes:
        groups.append(list(range(jt, jt + s)))
        jt += s

    if VARIANT == "asym":
        d2_cfg = [("d2a", 1, 4), ("d2b", 1, 3)]   # (name, bufs, banks)
        sacc_bufs = 1
    elif VARIANT == "g2b3":
        d2_cfg = [("d2", 3, 2)]
        sacc_bufs = 2
    else:
        d2_cfg = [("d2", 2, GROUP)]
        sacc_bufs = 2

    with tile.TileContext(nc) as tc:
        import contextlib
        with contextlib.ExitStack() as stack:
            cpool = stack.enter_context(tc.tile_pool(name="const", bufs=1))
            kpool = stack.enter_context(tc.tile_pool(name="ksb", bufs=3))
            spool = stack.enter_context(tc.tile_pool(name="ssb", bufs=2))
            d2pools = [
                (stack.enter_context(
                    tc.tile_pool(name=nm, bufs=bf, space="PSUM")), banks)
                for nm, bf, banks in d2_cfg
            ]
            sapool = stack.enter_context(
                tc.tile_pool(name="sacc", bufs=sacc_bufs, space="PSUM"))
            if packed:
                # bands of 13 weight rows at partitions {0,32,64} so the
                # three gen matmuls of a group occupy disjoint PE row
                # groups and run concurrently (tile_position auto-derives
                # from base_partition).
                a_sb = cpool.tile([64 + KDIM, N], dt.float16)
                b_sb = cpool.tile([64 + KDIM, RPC], dt.float16)
                for b in range(GROUP):
                    nc.sync.dma_start(
                        out=a_sb[32 * b:32 * b + KDIM, :],
                        in_=Ah.ap()[KDIM * b:KDIM * (b + 1), :])
                    nc.sync.dma_start(
                        out=b_sb[32 * b:32 * b + KDIM, :],
                        in_=Bh.ap()[KDIM * b:KDIM * (b + 1), :])
            else:
                a_sb = cpool.tile([KDIM, N], dt.float16)
                b_sb = cpool.tile([KDIM, RPC], dt.float16)
                nc.sync.dma_start(out=a_sb, in_=Ah.ap())
                nc.sync.dma_start(out=b_sb, in_=Bh.ap())
            r_sb = cpool.tile([JTILE, NJT * RCOLS], dt.float16)
            nc.sync.dma_start(out=r_sb, in_=Rh.ap())

            for ib in [i % NIB for i in range(NIB * loop_m)]:
                s_ps = sapool.tile([RCOLS, ICHUNK], dt.float32)
                isl = slice(ib * ICHUNK, (ib + 1) * ICHUNK)

                pending = None  # (jts, k_sb) whose reduction is not yet emitted
                for gi, jts in enumerate(groups):
                    w = len(jts) * ICHUNK
                    pool, banks = d2pools[gi % len(d2pools)]
                    d2 = pool.tile([JTILE, banks * ICHUNK], dt.float32)
                    for idx, jt in enumerate(jts):
                        if packed:
                            lhsT = a_sb[32 * idx:32 * idx + KDIM,
                                        jt * JTILE:(jt + 1) * JTILE]
                            rhs = b_sb[32 * idx:32 * idx + KDIM, isl]
                        else:
                            lhsT = a_sb[:, jt * JTILE:(jt + 1) * JTILE]
                            rhs = b_sb[:, isl]
                        nc.tensor.matmul(
                            d2[:, idx * ICHUNK:(idx + 1) * ICHUNK],
                            lhsT, rhs,
                            start=True, stop=True,
                        )
                    # software pipeline: emit previous group's reductions
                    # between this group's gen and exp so the PE never FIFO
                    # blocks behind a reduction waiting on the ACT engine.
                    if pending is not None and VARIANT not in ("gen", "genact"):
                        pjts, pk = pending
                        for idx, jt in enumerate(pjts):
                            nc.tensor.matmul(
                                s_ps,
                                r_sb[:, jt * RCOLS:(jt + 1) * RCOLS],
                                pk[:, idx * ICHUNK:(idx + 1) * ICHUNK],
                                start=(jt == 0), stop=(jt == NJT - 1),
                            )
                    if VARIANT != "gen":
                        k_sb = kpool.tile([JTILE, banks * ICHUNK], dt.float16)
                        nc.scalar.activation(
                            k_sb[:, :w], d2[:, :w],
                            mybir.ActivationFunctionType.Exp,
                            scale=-1.0 / (2.0 * SIG2),
                        )
                        pending = (jts, k_sb)

                if VARIANT not in ("gen", "genact"):
                    pjts, pk = pending
                    for idx, jt in enumerate(pjts):
                        nc.tensor.matmul(
                            s_ps,
                            r_sb[:, jt * RCOLS:(jt + 1) * RCOLS],
                            pk[:, idx * ICHUNK:(idx + 1) * ICHUNK],
                            start=(jt == 0), stop=(jt == NJT - 1),
                        )

                s_out = spool.tile([RCOLS, ICHUNK], dt.float32)
                if VARIANT not in ("gen", "genact"):
                    nc.vector.tensor_copy(s_out, s_ps)
                else:
                    nc.vector.memset(s_out, 0.0)
                nc.sync.dma_start(
                    out=So.ap()[:, ib * ICHUNK:(ib + 1) * ICHUNK], in_=s_out
                )

    nc.compile()
    return nc


def _build_program_sym(loop_m: int = 1):
    """Symmetric block-pair program (one SPMD program for all 8 cores).

    Core c inputs (host pre-gathers so the program is core-agnostic):
      a_gen [13, 5*BLK]  fp16  gen lhsT columns for j-blocks (c+s)%8, s=0..4
      b_gen [13, BLK]    fp16  gen rhs columns for i-block c
      r_red [128, 5*8*12] fp16 R chunks per (slot, j-tile)
    Outputs:
      s_fwd [12, BLK]    f32   rows of block c (accumulated over slots 0..4)
      s_mir [12, 3*BLK]  f32   partial rows of blocks c+1..c+3 (slots 1..3)

    Per slot: 16 gen matmuls ([128j x 512i], fp16 hi/lo trick), exp on ACT
    (groups of 3/3/2 PSUM banks), forward reduce into PSUM accumulators.
    Mirror slots additionally XBAR-DMA-transpose each K[jt] [128,1024] into
    T[jt] = [128 i_in, 8 ib, 128 j] and reduce over i with R_c as lhsT.
    All mirror work is queued and drained during the NEXT slot so the PE
    never blocks on the transpose DMAs.
    """
    import concourse.mybir as mybir
    import concourse.tile as tile
    from concourse import bacc

    dt = mybir.dt
    nc = bacc.Bacc("TRN2", target_bir_lowering=False, debug=False)

    Ah = nc.dram_tensor("a_gen", [KDIM, SLOTS * BLK], dt.float16,
                        kind="ExternalInput")
    Bh = nc.dram_tensor("b_gen", [KDIM, BLK], dt.float16,
                        kind="ExternalInput")
    Rh = nc.dram_tensor("r_red", [JTILE, SLOTS * NJT_B * RCOLS], dt.float16,
                        kind="ExternalInput")
    Sf = nc.dram_tensor("s_fwd", [RCOLS, BLK], dt.float32,
                        kind="ExternalOutput")
    Sm = nc.dram_tensor("s_mir", [RCOLS, len(MIR_SLOTS) * BLK], dt.float32,
                        kind="ExternalOutput")

    # exp group plan per slot, ic-major: (ic, jts) with sizes 3/3/2
    GROUPS = []
    for ic in range(2):
        for jts in ([0, 1, 2], [3, 4, 5], [6, 7]):
            GROUPS.append((ic, list(jts)))

    with tile.TileContext(nc) as tc:
        import contextlib
        with contextlib.ExitStack() as stack:
            cpool = stack.enter_context(tc.tile_pool(name="const", bufs=1))
            kpool = stack.enter_context(tc.tile_pool(name="ksb", bufs=2))
            tpool = stack.enter_context(tc.tile_pool(name="tsb", bufs=2))
            spool = stack.enter_context(tc.tile_pool(name="ssb", bufs=2))
            d2pool = stack.enter_context(
                tc.tile_pool(name="d2", bufs=2, space="PSUM"))
            sapool = stack.enter_context(
                tc.tile_pool(name="sacc", bufs=1, space="PSUM"))

            a_sb = cpool.tile([KDIM, SLOTS * BLK], dt.float16)
            b_sb = cpool.tile([KDIM, BLK], dt.float16)
            r_sb = cpool.tile([JTILE, SLOTS * NJT_B * RCOLS], dt.float16)
            nc.sync.dma_start(out=a_sb, in_=Ah.ap())
            nc.sync.dma_start(out=b_sb, in_=Bh.ap())
            nc.sync.dma_start(out=r_sb, in_=Rh.ap())

            # two PSUM banks hold all four accumulators on disjoint
            # partition ranges (matmul output base partition must be 0/32/64)
            acc = sapool.tile([76, 2 * ICHUNK], dt.float32)
            acc_f = [acc[0:RCOLS, 0:ICHUNK],
                     acc[32:32 + RCOLS, 0:ICHUNK]]            # fwd ic0, ic1
            acc_m = [acc[64:64 + RCOLS, 0:ICHUNK],
                     acc[0:RCOLS, ICHUNK:2 * ICHUNK]]         # mirror h0, h1

            for it in range(loop_m):
                pe_q = []   # deferred mirror PE/DVE work from previous slot
                prev = None  # (slot, ic, jts, k_arena) not yet reduced fwd

                def emit_red_fwd(pslot, pic, pjts, pk):
                    for jt in pjts:
                        nc.tensor.matmul(
                            acc_f[pic],
                            r_sb[:, (pslot * NJT_B + jt) * RCOLS:
                                 (pslot * NJT_B + jt + 1) * RCOLS],
                            pk[:, jt * BLK + pic * ICHUNK:
                               jt * BLK + pic * ICHUNK + ICHUNK],
                            start=(pslot == 0 and jt == 0),
                            stop=(pslot == SLOTS - 1 and jt == NJT_B - 1),
                            skip_group_check=True,
                        )

                mir_sb = spool.tile([RCOLS, len(MIR_SLOTS) * BLK], dt.float32)

                for s in range(SLOTS):
                    k_arena = kpool.tile([JTILE, NJT_B * BLK], dt.float16)
                    for gi, (ic, jts) in enumerate(GROUPS):
                        g = len(jts)
                        d2 = d2pool.tile([JTILE, g * ICHUNK], dt.float32)
                        for idx, jt in enumerate(jts):
                            nc.tensor.matmul(
                                d2[:, idx * ICHUNK:(idx + 1) * ICHUNK],
                                a_sb[:, s * BLK + jt * JTILE:
                                     s * BLK + (jt + 1) * JTILE],
                                b_sb[:, ic * ICHUNK:(ic + 1) * ICHUNK],
                                start=True, stop=True,
                            )
                        # software pipeline: previous group's fwd reductions
                        if prev is not None:
                            emit_red_fwd(*prev)
                        # drain deferred mirror work (starts at group 2 so
                        # the transpose DMAs get a head start)
                        if gi >= 2:
                            for _ in range(5):
                                if pe_q:
                                    pe_q.pop(0)()
                        k_out = (
                            k_arena[:, jts[0] * BLK:(jts[-1] + 1) * BLK]
                            .rearrange("p (t x) -> p t x", t=g)
                            [:, :, ic * ICHUNK:(ic + 1) * ICHUNK]
                        )
                        nc.scalar.activation(
                            k_out,
                            d2.rearrange("p (t w) -> p t w", t=g),
                            mybir.ActivationFunctionType.Exp,
                            scale=-1.0 / (2.0 * SIG2),
                        )
                        prev = (s, ic, jts, k_arena)

                    # queue mirror work for this slot (drained next slot)
                    if s in MIR_SLOTS:
                        t_arena = tpool.tile([JTILE, NJT_B * BLK], dt.float16)
                        for jt in range(NJT_B):
                            # XBAR transposes are SP-engine: issue now, they
                            # run in the DMA background
                            nc.sync.dma_start_transpose(
                                t_arena[:, jt * BLK:(jt + 1) * BLK]
                                .rearrange("p (b j) -> p b j", j=JTILE),
                                k_arena[:, jt * BLK:(jt + 1) * BLK],
                            )

                        def q_mir(ms, ta):
                            for h in range(2):
                                for ib in range(NJT_B):
                                    def red_j(h=h, ib=ib, ta=ta):
                                        rhs = (
                                            ta[:, h * 4 * BLK:(h * 4 + 4) * BLK]
                                            .rearrange("p (t x) -> p t x", t=4)
                                            [:, :, ib * JTILE:(ib + 1) * JTILE]
                                        )
                                        nc.tensor.matmul(
                                            acc_m[h],
                                            r_sb[:, ib * RCOLS:(ib + 1) * RCOLS],
                                            rhs,
                                            start=(ib == 0),
                                            stop=(ib == NJT_B - 1),
                                            skip_group_check=True,
                                        )
                                    pe_q.append(red_j)

                                def copy_out(ms=ms, h=h):
                                    nc.vector.tensor_copy(
                                        mir_sb[:, (ms - 1) * BLK + h * ICHUNK:
                                               (ms - 1) * BLK + (h + 1) * ICHUNK],
                                        acc_m[h],
                                    )
                                pe_q.append(copy_out)

                        q_mir(s, t_arena)

                emit_red_fwd(*prev)
                prev = None
                while pe_q:
                    pe_q.pop(0)()

                fwd_sb = spool.tile([RCOLS, BLK], dt.float32)
                for ic in range(2):
                    nc.vector.tensor_copy(
                        fwd_sb[:, ic * ICHUNK:(ic + 1) * ICHUNK], acc_f[ic])
                nc.sync.dma_start(out=Sf.ap(), in_=fwd_sb)
                nc.sync.dma_start(out=Sm.ap(), in_=mir_sb)

    nc.compile()
    return nc


def _split_hi_lo(v32: np.ndarray):
    """fp32 -> (hi, lo) float16 pair with v ~= hi + lo."""
    hi = v32.astype(np.float16)
    lo = (v32 - hi.astype(np.float32)).astype(np.float16)
    return hi, lo


def _host_prep(mom: np.ndarray, control_points: np.ndarray):
    x = np.asarray(control_points, np.float32).reshape(N, D)
    p = np.clip(np.asarray(mom, np.float32).reshape(N, D), -1.0, 1.0)

    sq = np.sum(x.astype(np.float64) * x.astype(np.float64), axis=1)
    sq = sq.astype(np.float32)
    xh, xl = _split_hi_lo(x)
    sqh, sql = _split_hi_lo(sq)
    ones = np.ones(N, np.float16)

    # lhsT (stationary, per-j): 13 rows
    A = np.empty((KDIM, N), np.float16)
    A[0:3] = xh.T
    A[3:6] = xl.T
    A[6:9] = xh.T
    A[9] = sqh
    A[10] = sql
    A[11] = ones
    A[12] = ones

    # rhs (moving, per-i): 13 rows
    m2xh = (-2.0 * xh.astype(np.float32)).astype(np.float16)
    m2xl = (-2.0 * xl.astype(np.float32)).astype(np.float16)
    Bfull = np.empty((KDIM, N), np.float16)
    Bfull[0:3] = m2xh.T
    Bfull[3:6] = m2xh.T
    Bfull[6:9] = m2xl.T
    Bfull[9] = ones
    Bfull[10] = ones
    Bfull[11] = sqh
    Bfull[12] = sql

    # reduction matrix R = [p | vec(p (x) x)], packed [128, 64*12]
    R = np.empty((N, RCOLS), np.float32)
    R[:, 0:3] = p
    R[:, 3:12] = (p[:, :, None] * x[:, None, :]).reshape(N, 9)
    Rp = (
        R.reshape(NJT, JTILE, RCOLS)
        .transpose(1, 0, 2)
        .reshape(JTILE, NJT * RCOLS)
        .astype(np.float16)
    )
    return x, p, A, Bfull, Rp


def build_in_maps_sym(A, Bfull, R16):
    """Per-core gathered operands for the symmetric schedule."""
    in_maps = []
    for c in range(NCORES):
        blocks = [(c + s) % NBLK for s in range(SLOTS)]
        a_gen = np.concatenate(
            [A[:, b * BLK:(b + 1) * BLK] for b in blocks], axis=1)
        b_gen = Bfull[:, c * BLK:(c + 1) * BLK]
        r_red = np.empty((JTILE, SLOTS * NJT_B * RCOLS), np.float16)
        for s, b in enumerate(blocks):
            for jt in range(NJT_B):
                rows = slice(b * BLK + jt * JTILE, b * BLK + (jt + 1) * JTILE)
                r_red[:, (s * NJT_B + jt) * RCOLS:
                      (s * NJT_B + jt + 1) * RCOLS] = R16[rows]
        in_maps.append({
            "a_gen": np.ascontiguousarray(a_gen),
            "b_gen": np.ascontiguousarray(b_gen),
            "r_red": r_red,
        })
    return in_maps


def assemble_S_sym(results):
    """Sum per-core s_fwd/s_mir partials into the full S [12, N]."""
    S = np.zeros((RCOLS, N), np.float64)
    for c in range(NCORES):
        S[:, c * BLK:(c + 1) * BLK] += results[c]["s_fwd"]
        for di, d in enumerate(MIR_SLOTS):
            b = (c + d) % NBLK
            S[:, b * BLK:(b + 1) * BLK] += \
                results[c]["s_mir"][:, di * BLK:(di + 1) * BLK]
    return S.astype(np.float32)


def build_in_maps(A, Bfull, Rp):
    nband = GROUP if VARIANT == "packed" else 1
    At = np.ascontiguousarray(np.tile(A, (nband, 1)))
    in_maps = []
    for c in range(NCORES):
        Bc = Bfull[:, c * RPC:(c + 1) * RPC]
        in_maps.append({
            "a_gen": At,
            "b_gen": np.ascontiguousarray(np.tile(Bc, (nband, 1))),
            "r_red": Rp,
        })
    return in_maps


def kernel(mom: np.ndarray, control_points: np.ndarray):
    global last_result
    from concourse.bass_utils import run_bass_kernel_spmd

    x, p, A, Bfull, Rp = _host_prep(mom, control_points)

    loop_m = int(os.environ.get("KERNEL_LOOP_M", "1"))
    key = ("nc", loop_m, VARIANT)

    if VARIANT == "sym":
        if key not in _cache:
            _cache[key] = _build_program_sym(loop_m)
        nc = _cache[key]
        R = np.empty((N, RCOLS), np.float32)
        R[:, 0:3] = p
        R[:, 3:12] = (p[:, :, None] * x[:, None, :]).reshape(N, 9)
        in_maps = build_in_maps_sym(A, Bfull, R.astype(np.float16))
        trace = os.environ.get("KERNEL_TRACE", "0") == "1"
        res = run_bass_kernel_spmd(
            nc, in_maps, core_ids=list(range(NCORES)), trace=trace,
        )
        last_result = res
        S = assemble_S_sym(res.results)
        dcp = S[0:3].T
        row = np.einsum("nd,dn->n", p, S[0:3])
        Wx = np.einsum("nd,den->ne", p, S[3:12].reshape(D, D, N))
        dmom = (1.0 / SIG2) * (x * row[:, None] - Wx)
        return (
            dmom.reshape(1, N, D).astype(np.float32),
            dcp.reshape(1, N, D).astype(np.float32),
        )

    if key not in _cache:
        _cache[key] = _build_program(loop_m)
    nc = _cache[key]

    in_maps = build_in_maps(A, Bfull, Rp)

    trace = os.environ.get("KERNEL_TRACE", "0") == "1"
    res = run_bass_kernel_spmd(
        nc, in_maps, core_ids=list(range(NCORES)), trace=trace,
    )
    last_result = res

    S = np.concatenate([r["s_out"] for r in res.results], axis=1)  # [12, N]

    dcp = S[0:3].T                                   # [N, 3]
    row = np.einsum("nd,dn->n", p, S[0:3])           # p_i . (K p)_i
    Wx = np.einsum("nd,den->ne", p, S[3:12].reshape(D, D, N))
    dmom = (1.0 / SIG2) * (x * row[:, None] - Wx)

    return (
        dmom.reshape(1, N, D).astype(np.float32),
        dcp.reshape(1, N, D).astype(np.float32),
    )

